# revision 8
# baseline (speedup 1.0000x reference)
"""Trainium2 Bass kernel for nn_Polo_AttentionTransNet.

Network (per sample):
  out branch: conv5x5(24->50) +b, relu, maxpool2   (24,8,32) -> (50,4,16)
              conv5x5(50->100)+b, relu, maxpool2   -> (100,2,8)
  in  branch: conv5x5(24->50) +b, relu, maxpool2   (24,4,16) -> (50,2,8)
  concat channels -> (150,2,8) -> flatten 2400 -> fc1(500)+relu -> fc2(2)
  Plus constant outputs theta (identity affine) and z (zeros).

The big `x` input (B,3,256,256) is unused by the reference computation
(only its batch size matters), so it is never sent to the device.

Strategy: pure data parallelism over 8 NeuronCores (32 samples each).
Each conv becomes 5 accumulating TensorE matmuls (one per kernel row dy)
with contraction K = 5*in_ch (kernel cols dx folded into partitions).
The conv input lives in SBUF replicated into 5 dx-shifted partition
groups over a zero-padded (rows+cols) per-sample layout; because every
group shares one flat layout, each dx replica is a single contiguous
copy at element offset (dx-2). Network inputs are pre-padded on the
host so their replicas stream straight from HBM; the conv1 output is
replicated by flat SBUF->SBUF copies. 2x2 maxpool = two strided
tensor_max ops on DVE; relu+bias fused into the ScalarE PSUM->SBUF
copy (bias commutes with max). fc1 contracts over (channel,hw) via an
HBM-bounce transpose of the pooled features; fc2 via PE-transpose of
fc1's output. fc biases fold in as ones-rows of the contraction.
"""

import numpy as np

import concourse.bass as bass
import concourse.mybir as mybir
import concourse.tile as tile
from concourse.ap import AP
from concourse.bass_utils import run_bass_kernel_spmd
from concourse.masks import make_identity
from concourse.tile import ScopedClock

FP32 = mybir.dt.float32
RELU = mybir.ActivationFunctionType.Relu

NCORES = 8
B = 256
BPC = B // NCORES  # 32 samples per core

# conv1 padded layout: per-sample 10 rows (8 data + 2 shared zero pad)
# of width 34 (2 zero cols at cc 0,1; data at cc [2,34)), 2 lead rows.
C1_W = 34
C1_SP = 340
C1_F = 10952  # 322 rows read * 34 + slack
# conv2(a/b) layout: per-sample 6 rows (4 data + 2 pad) of width 18.
C2_W = 18
C2_SP = 108
C2_F = 3496  # 194 rows * 18 + slack
# conv2a partition-group -> dx mapping (canonical dx=2 lives in group 0 so
# the conv1 epilogue writes at an aligned partition base)
DXS = (2, 0, 1, 3, 4)


class PatchedTC(tile.TileContext):
    """TileContext whose kernel-tail drain splits sem waits.

    The walrus build in this container rejects >1 sync wait on a
    CTRL-queue instruction (Drain); Tile's exit puts every outstanding
    proc's wait on one Drain. Split them across a chain of Drains.
    """

    def _drain_and_barrier(self, tick_clock, wait_clock):
        drain_inst = self.nc.sync.drain()
        wait_clock.add_sem_waits(
            drain_inst.ins, ScopedClock({None: tick_clock.global_clock})
        )
        si = drain_inst.ins.sync_info
        if si is not None and len(si.on_wait) > 1:
            waits = list(si.on_wait)
            drain_inst.ins.sync_info = mybir.SyncInfo(
                on_wait=waits[:1], on_update=list(si.on_update)
            )
            for i in range(1, len(waits)):
                extra = self.nc.sync.drain()
                extra.ins.sync_info = mybir.SyncInfo(
                    on_wait=waits[i : i + 1], on_update=[]
                )
        self.nc.all_engine_barrier()
        popped = self.nc._tile_sem_poison_stack.pop()
        assert popped is self._sem_poison
        self.nc.clear_and_free_semaphores(list(self.sems.allocated().values()))
        self.nc.all_engine_barrier()


def _sb(tile_ap, p0, np_, off, dims):
    """Strided view of an SBUF tile: partitions [p0, p0+np_), free dims
    [[step, count], ...] from flat free offset `off`."""
    t = tile_ap.tensor
    f = t.shape[1]
    return AP(tensor=t, offset=p0 * f + off, ap=[[f, np_]] + [list(d) for d in dims])




_WAIT_LIMIT = 1  # walrus in this container: max sync waits per instruction


def _split_bir_waits(bir_bytes):
    """Split >_WAIT_LIMIT sem waits onto NoOps preceding the instruction
    (same engine queue), which this walrus build requires."""
    import json as _json

    d = _json.loads(bir_bytes)
    ctr = [0]
    for fn in d["functions"]:
        for blk in fn["blocks"]:
            out = []
            for ins in blk["instructions"]:
                si = ins.get("sync_info")
                if si and len(si.get("on_wait") or []) > _WAIT_LIMIT:
                    waits = si["on_wait"]
                    for w in waits[:-_WAIT_LIMIT]:
                        ctr[0] += 1
                        out.append(
                            {
                                "debug": 0,
                                "engine": ins["engine"],
                                "ins": [],
                                "name": f"{ins['name']}-sw{ctr[0]}",
                                "opcode": "NoOp",
                                "outs": [],
                                "sync_info": {"on_update": [], "on_wait": [w]},
                            }
                        )
                    si["on_wait"] = waits[-_WAIT_LIMIT:]
                out.append(ins)
            blk["instructions"] = out
    return _json.dumps(d).encode()


def _install_wait_splitter(nc):
    orig = nc.to_json_bytes
    nc.to_json_bytes = lambda: _split_bir_waits(orig())
    return nc


def build_nc(debug=False):
    nc = bass.Bass()
    # host-padded conv inputs (canonical dx=2 layout, zeros elsewhere)
    xo = nc.dram_tensor("xo", (24, C1_F), FP32, kind="ExternalInput")
    xi = nc.dram_tensor("xi", (24, C2_F), FP32, kind="ExternalInput")
    w1 = nc.dram_tensor("w1", (5, 120, 57), FP32, kind="ExternalInput")
    w2a = nc.dram_tensor("w2a", (10, 125, 100), FP32, kind="ExternalInput")
    w2b = nc.dram_tensor("w2b", (5, 120, 50), FP32, kind="ExternalInput")
    bias1 = nc.dram_tensor("bias1", (57, 1), FP32, kind="ExternalInput")
    bias2a = nc.dram_tensor("bias2a", (100, 1), FP32, kind="ExternalInput")
    bias2b = nc.dram_tensor("bias2b", (50, 1), FP32, kind="ExternalInput")
    wfc1 = nc.dram_tensor("wfc1", (19, 128, 500), FP32, kind="ExternalInput")
    wfc2 = nc.dram_tensor("wfc2", (4, 126, 2), FP32, kind="ExternalInput")
    yt = nc.dram_tensor("yt", (2, BPC), FP32, kind="ExternalOutput")
    # fc feature scratch, layout (c, hw, b); padded to 152 rows so the
    # last 128-row fc1 chunk's load stays in bounds.
    scr_kind = "ExternalOutput" if debug else "Internal"
    scratch = nc.dram_tensor("feat", (152, 16, BPC), FP32, kind=scr_kind)
    if debug:
        dbg_p1a = nc.dram_tensor("dbg_p1a", (25, C2_F), FP32, kind="ExternalOutput")
        dbg_p1b = nc.dram_tensor("dbg_p1b", (25, C2_F), FP32, kind="ExternalOutput")
        dbg_r1 = nc.dram_tensor("dbg_r1", (BPC, 500), FP32, kind="ExternalOutput")

    with PatchedTC(nc) as tc:
        with (
            tc.tile_pool(name="const", bufs=1) as const,
            tc.tile_pool(name="cps", bufs=4, space="PSUM") as cps,
            tc.tile_pool(name="fps", bufs=1, space="PSUM") as fps,
            tc.tile_pool(name="ep", bufs=4) as ep,
        ):
            # ---- conv1 input: 5 dx-shifted contiguous loads ----
            # (emitted first so the SP HWDGE ring starts them immediately;
            # split in b-halves to pipeline with the first matmuls)
            P1 = const.tile([128, C1_F], FP32)
            HB1 = 16 * C1_SP + 2 * C1_W  # flat span: first 16 samples + pads
            for bh in range(2):
                lo = 2 if bh == 0 else HB1
                hi = HB1 if bh == 0 else C1_F - 4
                for dx in range(5):
                    nc.sync.dma_start(
                        P1[24 * dx : 24 * dx + 24, lo:hi],
                        xo[:, lo + dx - 2 : hi + dx - 2],
                    )

            # ---- conv2b input: 5 dx-shifted contiguous loads ----
            P2B = const.tile([128, C2_F], FP32)
            for dx in range(5):
                nc.scalar.dma_start(
                    P2B[24 * dx : 24 * dx + 24, 2 : C2_F - 4],
                    xi[:, dx : C2_F - 6 + dx],
                )

            # ---- weight/bias loads (ACT HWDGE ring) ----
            W1 = const.tile([120, 5 * 57], FP32)
            nc.scalar.dma_start(
                W1[:].rearrange("p (d o) -> p d o", d=5), w1[:].transpose([1, 0, 2])
            )
            W2B = const.tile([120, 250], FP32)
            nc.scalar.dma_start(
                W2B[:].rearrange("p (d o) -> p d o", d=5), w2b[:].transpose([1, 0, 2])
            )
            W2A = const.tile([125, 1000], FP32)
            nc.scalar.dma_start(
                W2A[:].rearrange("p (d o) -> p d o", d=10), w2a[:].transpose([1, 0, 2])
            )
            BI1 = const.tile([57, 1], FP32)
            nc.scalar.dma_start(BI1[:], bias1[:])
            BI2A = const.tile([100, 1], FP32)
            nc.scalar.dma_start(BI2A[:], bias2a[:])
            BI2B = const.tile([50, 1], FP32)
            nc.scalar.dma_start(BI2B[:], bias2b[:])
            WFC1 = const.tile([128, 19 * 500], FP32)
            nc.scalar.dma_start(
                WFC1[:].rearrange("p (q o) -> p q o", q=19), wfc1[:].transpose([1, 0, 2])
            )
            WFC2 = const.tile([126, 8], FP32)
            nc.scalar.dma_start(
                WFC2[:].rearrange("p (q o) -> p q o", q=4), wfc2[:].transpose([1, 0, 2])
            )
            ident = const.tile([32, 32], FP32)
            make_identity(nc, ident[:])

            # ---- conv2a input tiles: zero the canonical (dx=2) group; the
            # flat-shift replication then propagates zeros everywhere ----
            A2A = const.tile([128, C2_F], FP32)
            B2A = const.tile([128, C2_F], FP32)
            for t in (A2A, B2A):
                # pad rows {6g, 6g+1}
                nc.vector.memset(
                    _sb(t[:], 0, 25, 0, [[C2_SP, 33], [1, 2 * C2_W]]), 0.0
                )
                # cc {0,1} of data rows
                nc.vector.memset(
                    _sb(t[:], 0, 25, 2 * C2_W, [[C2_SP, 32], [C2_W, 4], [1, 2]]), 0.0
                )
                # tail slack
                nc.vector.memset(t[0:25, C2_F - 4 : C2_F], 0.0)

            # ---- conv1: 16 chunks of 2 samples; 5 dy-matmuls each ----
            for ch in range(16):
                b0 = ch * 2
                ps = cps.tile([57, 512], FP32, tag="convps")
                for dy in range(5):
                    rhs = _sb(
                        P1[:], 0, 120,
                        (10 * b0 + dy) * C1_W + 2,
                        [[C1_SP, 2], [C1_W, 8], [1, 32]],
                    )
                    nc.tensor.matmul(
                        ps[:], W1[:, dy * 57 : dy * 57 + 57], rhs,
                        start=(dy == 0), stop=(dy == 4),
                    )
                # relu(conv + bias) off PSUM, then maxpool on SBUF
                s = ep.tile([57, 512], FP32)
                nc.scalar.activation(s[:], ps[:], RELU, bias=BI1[:])
                t1 = ep.tile([57, 256], FP32)
                w0 = _sb(s[:], 0, 57, 0, [[256, 2], [32, 8], [2, 16]])
                w1v = _sb(s[:], 0, 57, 1, [[256, 2], [32, 8], [2, 16]])
                nc.vector.tensor_max(t1[:], w0, w1v)
                # h-pair max straight into the canonical groups of A2A/B2A
                dims = [[C2_SP, 2], [C2_W, 4], [1, 16]]
                dstA = _sb(A2A[:], 0, 25, (2 + 6 * b0) * C2_W + 2, dims)
                dstB = _sb(B2A[:], 0, 25, (2 + 6 * b0) * C2_W + 2, dims)
                hA0 = _sb(t1[:], 0, 25, 0, [[128, 2], [32, 4], [1, 16]])
                hA1 = _sb(t1[:], 0, 25, 16, [[128, 2], [32, 4], [1, 16]])
                nc.vector.tensor_max(dstA, hA0, hA1)
                hB0 = _sb(t1[:], 32, 25, 0, [[128, 2], [32, 4], [1, 16]])
                hB1 = _sb(t1[:], 32, 25, 16, [[128, 2], [32, 4], [1, 16]])
                nc.vector.tensor_max(dstB, hB0, hB1)

            # replicate conv1 output into the other 4 dx groups: flat
            # shifted SBUF->SBUF copies (zeros propagate from the center)
            for t in (A2A, B2A):
                for g in (1, 2, 3, 4):
                    dx = DXS[g]
                    nc.sync.dma_start(
                        t[25 * g : 25 * g + 25, 2 : C2_F - 4],
                        t[0:25, dx : C2_F - 6 + dx],
                    )
                if debug:
                    nc.sync.dma_start(
                        (dbg_p1a if t is A2A else dbg_p1b)[:], t[0:25, :]
                    )

            # ---- conv2b: 4 chunks of 8 samples ----
            # epilogue writes features in (hw, b) order: cell = hw*32 + b
            FEATB = const.tile([50, 16 * BPC], FP32)
            for ch in range(4):
                b0 = ch * 8
                ps = cps.tile([50, 512], FP32, tag="convps")
                for dy in range(5):
                    rhs = _sb(
                        P2B[:], 0, 120,
                        (6 * b0 + dy) * C2_W + 2,
                        [[C2_SP, 8], [C2_W, 4], [1, 16]],
                    )
                    nc.tensor.matmul(
                        ps[:], W2B[:, dy * 50 : dy * 50 + 50], rhs,
                        start=(dy == 0), stop=(dy == 4),
                    )
                s = ep.tile([50, 512], FP32, tag="sb2")
                nc.scalar.activation(s[:], ps[:], RELU, bias=BI2B[:])
                t1 = ep.tile([50, 256], FP32, tag="t1b")
                w0 = _sb(s[:], 0, 50, 0, [[64, 8], [16, 4], [2, 8]])
                w1v = _sb(s[:], 0, 50, 1, [[64, 8], [16, 4], [2, 8]])
                nc.vector.tensor_max(t1[:], w0, w1v)
                # h-max into FEATB, free order (b8, h2, w8) -> steps (1, 256, 32)
                dst = _sb(FEATB[:], 0, 50, b0, [[1, 8], [256, 2], [32, 8]])
                h0 = _sb(t1[:], 0, 50, 0, [[32, 8], [16, 2], [1, 8]])
                h1 = _sb(t1[:], 0, 50, 8, [[32, 8], [16, 2], [1, 8]])
                nc.vector.tensor_max(dst, h0, h1)

            # ---- conv2a: 4 chunks of 8 samples; 10 matmuls each ----
            FEATA = const.tile([100, 16 * BPC], FP32)
            for ch in range(4):
                b0 = ch * 8
                ps = cps.tile([100, 512], FP32, tag="convps")
                for dy in range(5):
                    for half, t in enumerate((A2A, B2A)):
                        rhs = _sb(
                            t[:], 0, 125,
                            (6 * b0 + dy) * C2_W + 2,
                            [[C2_SP, 8], [C2_W, 4], [1, 16]],
                        )
                        nc.tensor.matmul(
                            ps[:],
                            W2A[:, (dy * 2 + half) * 100 : (dy * 2 + half) * 100 + 100],
                            rhs,
                            start=(dy == 0 and half == 0),
                            stop=(dy == 4 and half == 1),
                        )
                s = ep.tile([100, 512], FP32, tag="sa2")
                nc.scalar.activation(s[:], ps[:], RELU, bias=BI2A[:])
                t1 = ep.tile([100, 256], FP32, tag="t1a")
                w0 = _sb(s[:], 0, 100, 0, [[64, 8], [16, 4], [2, 8]])
                w1v = _sb(s[:], 0, 100, 1, [[64, 8], [16, 4], [2, 8]])
                nc.vector.tensor_max(t1[:], w0, w1v)
                dst = _sb(FEATA[:], 0, 100, b0, [[1, 8], [256, 2], [32, 8]])
                h0 = _sb(t1[:], 0, 100, 0, [[32, 8], [16, 2], [1, 8]])
                h1 = _sb(t1[:], 0, 100, 8, [[32, 8], [16, 2], [1, 8]])
                nc.vector.tensor_max(dst, h0, h1)

            # ---- fc feature transpose via HBM bounce ----
            nc.sync.dma_start(
                scratch[0:100].rearrange("c hw b -> c (hw b)"), FEATA[:]
            )
            nc.sync.dma_start(
                scratch[100:150].rearrange("c hw b -> c (hw b)"), FEATB[:]
            )
            FEAT = const.tile([128, 19 * BPC], FP32)
            scr_flat = scratch[:].rearrange("c hw b -> (c hw b)")
            src = AP(
                tensor=scr_flat.tensor, offset=0,
                ap=[[BPC, 128], [128 * BPC, 19], [1, BPC]],
            )
            nc.sync.dma_start(FEAT[:].rearrange("p (q b) -> p q b", q=19), src)
            # ones cell for the fc1 bias row (k=2400 -> chunk 18, row 96)
            nc.vector.memset(FEAT[96:97, 18 * BPC : 19 * BPC], 1.0)

            # ---- fc1: out[b, oc] accumulated over 19 K-chunks ----
            psf = fps.tile([32, 500], FP32, tag="fc1")
            for q in range(19):
                k = 128 if q < 18 else 97
                nc.tensor.matmul(
                    psf[:],
                    FEAT[0:k, q * BPC : (q + 1) * BPC],
                    WFC1[0:k, q * 500 : (q + 1) * 500],
                    start=(q == 0), stop=(q == 18),
                )
            R1 = const.tile([32, 500], FP32)
            nc.scalar.activation(R1[:], psf[:], RELU)
            if debug:
                nc.sync.dma_start(dbg_r1[:], R1[:])

            # ---- fc2: PE-transpose R1 into [oc, b], contract 4 chunks ----
            R1T = const.tile([126, 128], FP32)
            nc.vector.memset(R1T[96:126, :], 1.0)
            for q in range(4):
                pt = fps.tile([125, 32], FP32, tag="tr")
                nc.tensor.transpose(pt[:], R1[:, q * 125 : (q + 1) * 125], ident[:])
                nc.vector.tensor_copy(R1T[0:125, q * 32 : (q + 1) * 32], pt[:])
            psy = fps.tile([2, 32], FP32, tag="y")
            for q in range(4):
                k = 125 if q < 3 else 126
                nc.tensor.matmul(
                    psy[:],
                    WFC2[0:k, q * 2 : (q + 1) * 2],
                    R1T[0:k, q * 32 : (q + 1) * 32],
                    start=(q == 0), stop=(q == 3),
                )
            OY = const.tile([2, 32], FP32)
            nc.vector.tensor_copy(OY[:], psy[:])
            nc.sync.dma_start(yt[:], OY[:])

    return _install_wait_splitter(nc)


def pad_conv_input(x, sp_rows, width):
    """(b, c, h, w) -> (c, F) canonical padded layout: data rows at
    physical row 2 + sp_rows*b + r, data cols at cc [2, 2+w)."""
    b, c, h, w = x.shape
    f = C1_F if width == C1_W else C2_F
    out = np.zeros((c, f), np.float32)
    body = out[:, 2 * width : (2 + sp_rows * b) * width].reshape(c, b, sp_rows, width)
    body[:, :, :h, 2 : 2 + w] = x.transpose(1, 0, 2, 3)
    return out


def prep_weights(conv1_w, conv1_b, conv2a_w, conv2a_b, conv2b_w, conv2b_b,
                 fc1_w, fc1_b, fc2_w, fc2_b):
    """Host-side weight layout prep (shared by all cores)."""
    w1t = conv1_w.transpose(2, 3, 1, 0).reshape(5, 120, 50)
    w1 = np.zeros((5, 120, 57), np.float32)
    w1[:, :, 0:25] = w1t[:, :, 0:25]
    w1[:, :, 32:57] = w1t[:, :, 25:50]
    w2b = np.ascontiguousarray(
        conv2b_w.transpose(2, 3, 1, 0).reshape(5, 120, 50), dtype=np.float32
    )
    # (dy*2+half, g*25+j, oc) with partition group g holding dx=DXS[g]
    w2a = np.ascontiguousarray(
        conv2a_w.transpose(2, 3, 1, 0)[:, DXS]
        .reshape(5, 5, 2, 25, 100)
        .transpose(0, 2, 1, 3, 4)
        .reshape(10, 125, 100),
        dtype=np.float32,
    )
    wfc1 = np.zeros((19 * 128, 500), np.float32)
    wfc1[:2400] = fc1_w.T
    wfc1[2400] = fc1_b
    wfc1 = wfc1.reshape(19, 128, 500)
    wfc2 = np.zeros((4, 126, 2), np.float32)
    for q in range(4):
        wfc2[q, :125] = fc2_w.T[q * 125 : (q + 1) * 125]
    wfc2[3, 125] = fc2_b
    return {
        "w1": w1,
        "w2a": w2a,
        "w2b": w2b,
        "bias1": np.concatenate(
            [conv1_b[0:25], np.zeros(7, np.float32), conv1_b[25:50]]
        ).astype(np.float32).reshape(57, 1),
        "bias2a": np.asarray(conv2a_b, np.float32).reshape(100, 1),
        "bias2b": np.asarray(conv2b_b, np.float32).reshape(50, 1),
        "wfc1": np.ascontiguousarray(wfc1),
        "wfc2": np.ascontiguousarray(wfc2),
    }


_CACHE = {}


def _get_nc(debug=False):
    key = ("nc", debug)
    if key not in _CACHE:
        _CACHE[key] = build_nc(debug=debug)
    return _CACHE[key]


def run_cores(inputs, debug=False, trace=False):
    """Shard inputs, run the SPMD kernel on 8 cores, return results."""
    nc = _get_nc(debug=debug)
    w = prep_weights(
        inputs["conv1_w"], inputs["conv1_b"], inputs["conv2a_w"], inputs["conv2a_b"],
        inputs["conv2b_w"], inputs["conv2b_b"], inputs["fc1_w"], inputs["fc1_b"],
        inputs["fc2_w"], inputs["fc2_b"],
    )
    xo = np.asarray(inputs["x_polo_out"], np.float32)
    xi = np.asarray(inputs["x_polo_in"], np.float32)
    in_maps = []
    for c in range(NCORES):
        m = dict(w)
        m["xo"] = pad_conv_input(xo[c * BPC : (c + 1) * BPC], 10, C1_W)
        m["xi"] = pad_conv_input(xi[c * BPC : (c + 1) * BPC], 6, C2_W)
        in_maps.append(m)
    return run_bass_kernel_spmd(nc, in_maps, core_ids=list(range(NCORES)), trace=trace)


def kernel(**inputs):
    n = inputs["x"].shape[0]
    assert n == B, f"kernel hardcoded for batch {B}, got {n}"
    res = run_cores(inputs)
    y = np.concatenate([r["yt"].T for r in res.results], axis=0)
    theta = np.broadcast_to(
        np.array([[1.0, 0.0, 0.0], [0.0, 1.0, 0.0]], np.float32)[None], (n, 2, 3)
    ).copy()
    z = np.zeros((n, 2), np.float32)
    return (y.astype(np.float32), theta, z)
